# revision 6
# baseline (speedup 1.0000x reference)
"""GQA forward kernel for Trainium2 (8 NeuronCores, Bass/Tile).

Problem (hardcoded): B=2, S=2048, D=2048, H=16 q-heads, G=8 kv-groups, HD=128.

Sharding: tensor-parallel over heads. Core c owns q-heads {2c, 2c+1} and
kv-group c, for both batches. Each core computes a partial output
(its 256-dim slice of the concatenated context through its Wo rows);
the host sums the 8 partials.

Device-side dataflow per core (per batch):
  phase 1: QT/KT (option-B projections, [hd, s] layout, moving dim 512),
           V^T projection + PE transpose to [s, hd]; RMSNorm + RoPE fused
           into the evacuation ([hd, s] layout: rms-sum via ones-matmul
           broadcast, per-hd scales folded into host-prepared cos/sin).
  phase 2: scoresT[j, i] = kT.T @ qT (both operands natural), causal
           diag-mask add + Exp (ACT, PSUM->SBUF), ctxT accumulation
           lhsT=V tiles, rowsum via ones-matmul (replicated), divide on
           evacuation.
  phase 3: out[s, d] = ctxT.T @ Wo rows, partial over this core's heads.

All matmuls run in float32r (full PE rate at moving-dim>=256,
~1.6e-4 relative rounding).
"""

import numpy as np

import concourse.bacc as bacc
import concourse.mybir as mybir
import concourse.tile as tile
from concourse.bass_utils import run_bass_kernel_spmd

B, S, D = 2, 2048, 2048
H, G, HD = 16, 8, 128
EPS = 1e-6
N_CORES = 8
HPC = H // N_CORES          # q heads per core = 2
SR = 512                    # s-range width (moving dim)
NSR = S // SR               # 4
NDC = D // 128              # 16 contraction chunks for projections
NST = S // 128              # 16 s-tiles / j-chunks
NEG = -1.0e9                # additive causal mask value

F32 = mybir.dt.float32
F32R = mybir.dt.float32r

_CACHE = {}


def _build_nc():
    nc = bacc.Bacc("TRN2", target_bir_lowering=False, name="gqa")

    # ---- DRAM I/O (per-core tensors; same program on all 8 cores)
    d_xT = nc.dram_tensor("xT", [B, D, S], F32R, kind="ExternalInput")
    d_wq = nc.dram_tensor("wq", [128, NDC, HPC, HD], F32R, kind="ExternalInput")
    d_wk = nc.dram_tensor("wk", [128, NDC, HD], F32R, kind="ExternalInput")
    d_wv = nc.dram_tensor("wv", [128, NDC, HD], F32R, kind="ExternalInput")
    d_wo = nc.dram_tensor("wo", [128, HPC, D], F32R, kind="ExternalInput")
    d_cosq = nc.dram_tensor("cosq", [HD, S], F32, kind="ExternalInput")
    d_sinq = nc.dram_tensor("sinq", [HD, S], F32, kind="ExternalInput")
    d_cosk = nc.dram_tensor("cosk", [HD, S], F32, kind="ExternalInput")
    d_sink = nc.dram_tensor("sink", [HD, S], F32, kind="ExternalInput")
    d_ones = nc.dram_tensor("ones128", [128, 128], F32R, kind="ExternalInput")
    d_id = nc.dram_tensor("ident", [128, 128], F32R, kind="ExternalInput")
    d_tri = nc.dram_tensor("trimask", [128, 128], F32, kind="ExternalInput")
    d_out = nc.dram_tensor("out", [B, S, D], F32, kind="ExternalOutput")

    with tile.TileContext(nc) as tc:
        with (
            # resident SBUF
            tc.tile_pool(name="resident", bufs=1) as res_pool,
            tc.tile_pool(name="qkv", bufs=1) as qkv_pool,
            tc.tile_pool(name="ctx", bufs=1) as ctx_pool,
            # streamed SBUF
            tc.tile_pool(name="xt", bufs=18) as xt_pool,
            tc.tile_pool(name="tmp", bufs=2) as tmp_pool,
            tc.tile_pool(name="ew", bufs=3) as ew_pool,
            tc.tile_pool(name="ob", bufs=2) as ob_pool,
            # PSUM
            tc.tile_pool(name="ps_mm", bufs=2, space="PSUM") as ps_mm,
            tc.tile_pool(name="ps_ssum", bufs=1, space="PSUM") as ps_ssum,
            tc.tile_pool(name="ps_vt", bufs=1, space="PSUM") as ps_vt,
            tc.tile_pool(name="ps_sc", bufs=2, space="PSUM") as ps_sc,
            tc.tile_pool(name="ps_ctx", bufs=1, space="PSUM") as ps_ctx,
            tc.tile_pool(name="ps_rs", bufs=1, space="PSUM") as ps_rs,
        ):
            # ---- resident constants / weights
            wq_sb = res_pool.tile([128, NDC, HPC, HD], F32R)
            nc.sync.dma_start(out=wq_sb, in_=d_wq[:, :, :, :])
            wk_sb = res_pool.tile([128, NDC, HD], F32R)
            nc.sync.dma_start(out=wk_sb, in_=d_wk[:, :, :])
            wv_sb = res_pool.tile([128, NDC, HD], F32R)
            nc.sync.dma_start(out=wv_sb, in_=d_wv[:, :, :])
            wo_sb = res_pool.tile([128, HPC, D], F32R)
            nc.sync.dma_start(out=wo_sb, in_=d_wo[:, :, :])
            cosq_sb = res_pool.tile([HD, S], F32)
            nc.sync.dma_start(out=cosq_sb, in_=d_cosq[:, :])
            sinq_sb = res_pool.tile([HD, S], F32)
            nc.sync.dma_start(out=sinq_sb, in_=d_sinq[:, :])
            cosk_sb = res_pool.tile([HD, S], F32)
            nc.sync.dma_start(out=cosk_sb, in_=d_cosk[:, :])
            sink_sb = res_pool.tile([HD, S], F32)
            nc.sync.dma_start(out=sink_sb, in_=d_sink[:, :])
            ones_sb = res_pool.tile([128, 128], F32R)
            nc.sync.dma_start(out=ones_sb, in_=d_ones[:, :])
            id_sb = res_pool.tile([128, 128], F32R)
            nc.sync.dma_start(out=id_sb, in_=d_id[:, :])
            tri_sb = res_pool.tile([128, 128], F32)
            nc.sync.dma_start(out=tri_sb, in_=d_tri[:, :])
            eps_sb = res_pool.tile([128, 1], F32)
            nc.vector.memset(eps_sb, EPS)

            for b in range(B):
                # per-batch activations
                qT = [qkv_pool.tile([HD, S], F32R, tag=f"qT{h}",
                                    name=f"qT{h}_{b}") for h in range(HPC)]
                kT = qkv_pool.tile([HD, S], F32R, tag="kT")
                v_res = qkv_pool.tile([128, NST, HD], F32R, tag="v")
                ctxT = [ctx_pool.tile([HD, S], F32R, tag=f"ctxT{h}",
                                      name=f"ctxT{h}_{b}") for h in range(HPC)]

                # ======== phase 1: projections + rmsnorm + rope ========
                for sr in range(NSR):
                    ssl = slice(sr * SR, (sr + 1) * SR)
                    xts = []
                    for dc in range(NDC):
                        xt = xt_pool.tile([128, SR], F32R, tag="xt")
                        nc.sync.dma_start(
                            out=xt, in_=d_xT[b, dc * 128:(dc + 1) * 128, ssl])
                        xts.append(xt)

                    # q0, q1, k: rmsnorm + rope targets
                    for ti in range(HPC + 1):
                        ps = ps_mm.tile([128, SR], F32, tag="mm")
                        for dc in range(NDC):
                            w = (wq_sb[:, dc, ti, :] if ti < HPC
                                 else wk_sb[:, dc, :])
                            nc.tensor.matmul(ps, w, xts[dc],
                                             start=(dc == 0),
                                             stop=(dc == NDC - 1))
                        raw = tmp_pool.tile([128, SR], F32, tag="raw")
                        nc.vector.tensor_copy(out=raw, in_=ps)
                        sq = tmp_pool.tile([128, SR], F32R, tag="sq")
                        nc.scalar.activation(
                            out=sq, in_=ps,
                            func=mybir.ActivationFunctionType.Square)
                        ssum = ps_ssum.tile([128, SR], F32, tag="ssum")
                        nc.tensor.matmul(ssum, ones_sb, sq,
                                         start=True, stop=True)
                        rstd = tmp_pool.tile([128, SR], F32, tag="rstd")
                        nc.scalar.activation(
                            out=rstd, in_=ssum,
                            func=mybir.ActivationFunctionType.Sqrt,
                            bias=eps_sb[:, :], scale=1.0 / HD)
                        nc.vector.reciprocal(out=rstd, in_=rstd)
                        cosx = cosq_sb if ti < HPC else cosk_sb
                        sinx = sinq_sb if ti < HPC else sink_sb
                        m1 = tmp_pool.tile([128, SR], F32, tag="m1")
                        nc.vector.tensor_mul(m1, raw, cosx[:, ssl])
                        m2 = tmp_pool.tile([128, SR], F32, tag="m2")
                        nc.gpsimd.tensor_mul(m2[0:64, :], raw[64:128, :],
                                             sinx[64:128, ssl])
                        nc.gpsimd.tensor_mul(m2[64:128, :], raw[0:64, :],
                                             sinx[0:64, ssl])
                        nc.vector.tensor_add(m1, m1, m2)
                        dst = qT[ti] if ti < HPC else kT
                        nc.vector.tensor_mul(dst[:, ssl], m1, rstd)

                    # v: project vT then transpose to [s, hd]
                    ps = ps_mm.tile([128, SR], F32, tag="mm")
                    for dc in range(NDC):
                        nc.tensor.matmul(ps, wv_sb[:, dc, :], xts[dc],
                                         start=(dc == 0), stop=(dc == NDC - 1))
                    vt = tmp_pool.tile([128, SR], F32R, tag="vt")
                    nc.scalar.activation(
                        out=vt, in_=ps,
                        func=mybir.ActivationFunctionType.Copy)
                    for st in range(SR // 128):
                        pv = ps_vt.tile([128, 128], F32R, tag="vt")
                        nc.tensor.transpose(
                            pv, vt[:, st * 128:(st + 1) * 128], id_sb)
                        nc.vector.tensor_copy(
                            out=v_res[:, sr * 4 + st, :], in_=pv)

                # ======== phase 2: attention ========
                for h in range(HPC):
                    for r in range(NSR):
                        isl = slice(r * SR, (r + 1) * SR)
                        nt = 4 * r + 4
                        ctx_ps = ps_ctx.tile([128, SR], F32, tag="ctx")
                        rs_ps = ps_rs.tile([128, SR], F32, tag="rs")
                        for t in range(nt):
                            sc = ps_sc.tile([128, SR], F32, tag="sc")
                            nc.tensor.matmul(
                                sc, kT[:, t * 128:(t + 1) * 128],
                                qT[h][:, isl], start=True, stop=True)
                            ew = ew_pool.tile([128, SR], F32R, tag="ew")
                            co = (t - 4 * r) * 128 if t >= 4 * r else 0
                            if t >= 4 * r:
                                nc.vector.tensor_add(
                                    sc[:, co:co + 128],
                                    sc[:, co:co + 128], tri_sb)
                            nc.scalar.activation(
                                out=ew[:, co:], in_=sc[:, co:],
                                func=mybir.ActivationFunctionType.Exp)
                            nc.tensor.matmul(ctx_ps[:, co:], v_res[:, t, :],
                                             ew[:, co:],
                                             start=(t == 0),
                                             stop=(t == nt - 1))
                            nc.tensor.matmul(rs_ps[:, co:], ones_sb,
                                             ew[:, co:],
                                             start=(t == 0),
                                             stop=(t == nt - 1))
                        rcp = tmp_pool.tile([128, SR], F32, tag="rstd")
                        nc.vector.reciprocal(out=rcp, in_=rs_ps)
                        nc.vector.tensor_mul(ctxT[h][:, isl], ctx_ps, rcp)

                # ======== phase 3: output projection (partial) ========
                for dr in range(NSR):
                    dsl = slice(dr * SR, (dr + 1) * SR)
                    for st in range(NST):
                        op = ps_mm.tile([128, SR], F32, tag="mm")
                        for h in range(HPC):
                            nc.tensor.matmul(
                                op, ctxT[h][:, st * 128:(st + 1) * 128],
                                wo_sb[:, h, dsl],
                                start=(h == 0), stop=(h == HPC - 1))
                        ob = ob_pool.tile([128, SR], F32, tag="ob")
                        if st % 2 == 0:
                            nc.vector.tensor_copy(out=ob, in_=op)
                        else:
                            nc.scalar.activation(
                                out=ob, in_=op,
                                func=mybir.ActivationFunctionType.Copy)
                        nc.sync.dma_start(
                            out=d_out[b, st * 128:(st + 1) * 128, dsl],
                            in_=ob)

    nc.compile()
    return nc


def _prep_inputs(x, cos, sin, Wq, Wk, Wv, Wo, q_scale, k_scale):
    x = np.asarray(x, dtype=np.float32)
    cos = np.asarray(cos, dtype=np.float32)
    sin = np.asarray(sin, dtype=np.float32)
    Wq = np.asarray(Wq, dtype=np.float32)
    Wk = np.asarray(Wk, dtype=np.float32)
    Wv = np.asarray(Wv, dtype=np.float32)
    Wo = np.asarray(Wo, dtype=np.float32)
    qs = np.asarray(q_scale, dtype=np.float32)
    ks = np.asarray(k_scale, dtype=np.float32)

    xT = np.ascontiguousarray(x.transpose(0, 2, 1))  # (B, D, S)

    cosT = np.ascontiguousarray(cos.T)  # (HD, S)
    sinT = np.ascontiguousarray(sin.T)
    inv = np.float32(1.0 / np.sqrt(HD))
    hidx = np.arange(HD)
    rot = (hidx + HD // 2) % HD
    sign = np.where(hidx < HD // 2, -1.0, 1.0).astype(np.float32)[:, None]
    cosq = np.ascontiguousarray(cosT * qs[:, None] * inv)
    sinq = np.ascontiguousarray(
        np.roll(sinT * qs[rot][:, None] * sign * inv, -64, axis=0))
    cosk = np.ascontiguousarray(cosT * ks[:, None])
    sink = np.ascontiguousarray(
        np.roll(sinT * ks[rot][:, None] * sign, -64, axis=0))

    tri = np.where(np.arange(128)[:, None] > np.arange(128)[None, :],
                   np.float32(NEG), np.float32(0.0)).astype(np.float32)
    ident = np.eye(128, dtype=np.float32)
    ones128 = np.ones((128, 128), dtype=np.float32)

    shared = {
        "xT": xT, "cosq": cosq, "sinq": sinq, "cosk": cosk, "sink": sink,
        "ones128": ones128, "ident": ident, "trimask": tri,
    }
    in_maps = []
    for c in range(N_CORES):
        wq_c = Wq[:, c * HPC * HD:(c + 1) * HPC * HD]        # (D, HPC*HD)
        wq_c = wq_c.reshape(NDC, 128, HPC, HD).transpose(1, 0, 2, 3)
        wk_c = Wk[:, c * HD:(c + 1) * HD]                    # (D, HD)
        wk_c = wk_c.reshape(NDC, 128, HD).transpose(1, 0, 2)
        wv_c = Wv[:, c * HD:(c + 1) * HD]
        wv_c = wv_c.reshape(NDC, 128, HD).transpose(1, 0, 2)
        wo_c = Wo[c * HPC * HD:(c + 1) * HPC * HD, :]        # (HPC*HD, D)
        wo_c = wo_c.reshape(HPC, HD, D).transpose(1, 0, 2)
        m = dict(shared)
        m["wq"] = np.ascontiguousarray(wq_c)
        m["wk"] = np.ascontiguousarray(wk_c)
        m["wv"] = np.ascontiguousarray(wv_c)
        m["wo"] = np.ascontiguousarray(wo_c)
        in_maps.append(m)
    return in_maps


def _numpy_fallback(x, mask, cos, sin, Wq, Wk, Wv, Wo, q_scale, k_scale):
    """Bit-faithful numpy reimplementation (insurance for non-causal masks)."""
    x = np.asarray(x, np.float32)
    mask = np.asarray(mask)
    cos = np.asarray(cos, np.float32)
    sin = np.asarray(sin, np.float32)
    GS = H // G

    def rmsnorm(t, scale):
        ms = np.mean(t.astype(np.float32) ** 2, axis=-1, keepdims=True)
        return t * (1.0 / np.sqrt(ms + EPS)) * scale

    def rope(t):
        t1, t2 = t[..., :HD // 2], t[..., HD // 2:]
        rt = np.concatenate([-t2, t1], axis=-1)
        return t * cos[None, None, :, :] + rt * sin[None, None, :, :]

    q = (x @ Wq).reshape(B, S, H, HD).transpose(0, 2, 1, 3)
    k = (x @ Wk).reshape(B, S, G, HD).transpose(0, 2, 1, 3)
    v = (x @ Wv).reshape(B, S, G, HD).transpose(0, 2, 1, 3)
    q = rope(rmsnorm(q, np.asarray(q_scale, np.float32)))
    k = rope(rmsnorm(k, np.asarray(k_scale, np.float32)))
    out = np.zeros((B, S, H * HD), np.float32)
    for b in range(B):
        for g in range(G):
            for gi in range(GS):
                h = g * GS + gi
                sc = (q[b, h] @ k[b, g].T) / np.sqrt(np.float32(HD))
                sc = np.where(mask, -np.inf, sc)
                sc = sc - sc.max(axis=-1, keepdims=True)
                w = np.exp(sc)
                w /= w.sum(axis=-1, keepdims=True)
                out[b, :, h * HD:(h + 1) * HD] = w @ v[b, g]
    return (out @ np.asarray(Wo, np.float32)).astype(np.float32)


def kernel(x, mask, cos, sin, Wq, Wk, Wv, Wo, q_scale, k_scale):
    mask_np = np.asarray(mask)
    causal = np.array_equal(
        mask_np, np.triu(np.ones((S, S), dtype=bool), k=1))
    if not causal:
        return _numpy_fallback(x, mask, cos, sin, Wq, Wk, Wv, Wo,
                               q_scale, k_scale)

    in_maps = _prep_inputs(x, cos, sin, Wq, Wk, Wv, Wo, q_scale, k_scale)
    if "nc" not in _CACHE:
        _CACHE["nc"] = _build_nc()
    nc = _CACHE["nc"]
    res = run_bass_kernel_spmd(nc, in_maps, core_ids=list(range(N_CORES)))
    _CACHE["last_result"] = res
    out = res.results[0]["out"].astype(np.float32).copy()
    for c in range(1, N_CORES):
        out += res.results[c]["out"]
    return out
